# revision 1
# baseline (speedup 1.0000x reference)
"""MultiHeadCrossAttention kernel for 8 Trainium2 NeuronCores.

Reference computation (b=2, nq=nk=2048, d_model=512, h=8, hd=64):
    Q = split_heads(q @ Wq.T + bq); K, V likewise
    S = Q K^T * hd^-0.5 ; A = softmax(S, -1) * mask_head * diag(pearson)[k]
    out = merge_heads(A @ V)

Sharding: 16 (batch, head) pairs -> 2 heads of one batch per core.
Only the *diagonal* of pearson_matrix is used, so it is extracted on the
host (128 KiB instead of 128 MiB of device traffic) and folded into the
mask, which is also transposed on the host so the device kernel can work
entirely in a "k on partitions, q on free axis" layout:

    S^T[k,q]   = sum_d K^T[d,k] Q^T[d,q]           (TensorE, d=64 contraction)
    E^T        = exp(SCALING * S^T)                (ScalarE, PSUM->SBUF)
    Z[q]       = sum_k E^T[k,q]                    (TensorE, ones-vector lhsT)
    A^T        = E^T * maskT_folded                (VectorE, mask streamed from HBM)
    agg^T[e,q] = sum_k V[k,e] A^T[k,q]             (TensorE, accumulated over k tiles)
    out^T      = agg^T / Z                         (VectorE, Z partition-broadcast)

The device returns out^T (128 rows = 2 heads x 64 dims, 2048 cols = q);
the host transposes and concatenates the 8 per-core slices.
"""

import ctypes
import os
import sys
import types

import numpy as np

import concourse.bacc as bacc
import concourse.bass as bass
import concourse.tile as tile
from concourse import mybir
from concourse.vector_clock import ScopedClock

F32 = mybir.dt.float32

B = 2
H = 8
N = 2048  # nq == nk
D = 512
HD = 64
HPC = 2  # heads per core
E = HPC * HD  # 128 output dims per core
SCALING = HD ** (-0.5)
NCORES = 8
P = 128
QC = 1024  # q super-chunk (2 per core)
NQC = N // QC
NKT = N // P  # 16 k tiles


# ---------------------------------------------------------------------------
# Page faults are extremely slow in this sandbox (~ms each); MAP_POPULATE
# prefaults an allocation in one syscall, ~100x faster for big arrays.
# ---------------------------------------------------------------------------
_libc = ctypes.CDLL(None, use_errno=True)
_libc.mmap.restype = ctypes.c_void_p
_libc.mmap.argtypes = [
    ctypes.c_void_p,
    ctypes.c_size_t,
    ctypes.c_int,
    ctypes.c_int,
    ctypes.c_int,
    ctypes.c_long,
]


def _alloc(shape, dtype=np.float32):
    nbytes = int(np.prod(shape)) * np.dtype(dtype).itemsize
    nbytes = (nbytes + 4095) & ~4095
    p = _libc.mmap(None, nbytes, 0x3, 0x02 | 0x20 | 0x8000, -1, 0)  # RW, PRIV|ANON|POPULATE
    if p in (None, ctypes.c_void_p(-1).value):
        return np.empty(shape, dtype)
    buf = (ctypes.c_byte * nbytes).from_address(p)
    return np.frombuffer(buf, dtype=dtype, count=int(np.prod(shape))).reshape(shape)


def _tcopy(src):
    """Contiguous transposed copy of a 2-D array into prefaulted memory."""
    dst = _alloc((src.shape[1], src.shape[0]), src.dtype)
    np.copyto(dst, src.T)
    return dst


# ---------------------------------------------------------------------------
# Environment shim: walrus in this container rejects >1 sync wait on
# CTRL-class instructions (NoOp/Drain), but TileContext's kernel-tail drain
# carries one wait per live semaphore.  Re-emit them as individual wait_ge
# instructions (one wait each) before a bare drain.
# ---------------------------------------------------------------------------
def _drain_and_barrier(self, tick_clock, wait_clock):
    probe = mybir.InstNoOp(
        name="wait_probe", ins=[], outs=[], engine=mybir.EngineType.SP
    )
    wait_clock.add_sem_waits(probe, ScopedClock({None: tick_clock.global_clock}))
    waits = list(probe.sync_info.on_wait) if probe.sync_info else []
    allocated = self.sems.allocated()
    by_name = {}
    for k, h in allocated.items():
        by_name[getattr(h, "name", str(k))] = h
    for w in waits:
        h = by_name.get(w.ant_name)
        assert h is not None, (w.ant_name, sorted(by_name))
        self.nc.sync.wait_ge(h, w.wait_value)
    self.nc.sync.drain()
    self.nc.all_engine_barrier()
    popped = self.nc._tile_sem_poison_stack.pop()
    assert popped is self._sem_poison
    self.nc.clear_and_free_semaphores(list(allocated.values()))
    self.nc.all_engine_barrier()


def _install_shims():
    tile.TileContext._drain_and_barrier = _drain_and_barrier
    if "antenv.axon_hooks" not in sys.modules:
        try:
            from trn_agent_boot.trn_boot import _ntff_profile_via_ctypes

            mod = types.ModuleType("antenv.axon_hooks")
            hook = _ntff_profile_via_ctypes("/opt/axon/libaxon_pjrt.so")
            mod.get_axon_ntff_profile_hook = lambda: hook
            mod.set_axon_ntff_profile_hook = lambda h: None
            sys.modules["antenv.axon_hooks"] = mod
        except Exception:
            pass


# ---------------------------------------------------------------------------
# Device kernel (one Bass program, SPMD over 8 cores; shards via in_maps)
# ---------------------------------------------------------------------------
def build_nc() -> bass.Bass:
    # KERNEL_F32R: 0 = f32 everywhere; 1 = AV matmul in float32r;
    # 2 = also S^T in float32r.
    f32r_level = int(os.environ.get("KERNEL_F32R", "0"))
    R = mybir.dt.float32r
    AT_DT = R if f32r_level >= 1 else F32
    QK_DT = R if f32r_level >= 2 else F32
    HF = 512  # matmul half-width (one PSUM bank)
    NHF = QC // HF

    nc = bacc.Bacc("TRN2", target_bir_lowering=False, debug=False)

    PRJ_DT = R if f32r_level >= 2 else F32
    qT = nc.dram_tensor("qT", [D, N], PRJ_DT, kind="ExternalInput")
    kT = nc.dram_tensor("kT", [D, N], PRJ_DT, kind="ExternalInput")
    vT = nc.dram_tensor("vT", [D, N], PRJ_DT, kind="ExternalInput")
    wqT = nc.dram_tensor("wqT", [D, E], PRJ_DT, kind="ExternalInput")
    wkT = nc.dram_tensor("wkT", [D, E], PRJ_DT, kind="ExternalInput")
    wvT = nc.dram_tensor("wvT", [D, E], PRJ_DT, kind="ExternalInput")
    bq = nc.dram_tensor("bq", [E, 1], F32, kind="ExternalInput")
    bk = nc.dram_tensor("bk", [E, 1], F32, kind="ExternalInput")
    bv = nc.dram_tensor("bv", [1, E], F32, kind="ExternalInput")
    # maskT[lh, k, q] = mask[b, h0+lh, q, k] * diag(pearson)[b, h0+lh, k]
    maskT = nc.dram_tensor("maskT", [HPC, N, N], F32, kind="ExternalInput")
    outT = nc.dram_tensor("outT", [E, N], F32, kind="ExternalOutput")
    # softmax denominators, normalization happens on the host
    zout = nc.dram_tensor("zout", [HPC, N], F32, kind="ExternalOutput")

    ncc = D // P  # 4 contraction chunks for the projections

    with tile.TileContext(nc) as tc:
        with (
            tc.tile_pool(name="consts", bufs=1) as consts,
            tc.tile_pool(name="persist", bufs=1) as persist,
            tc.tile_pool(name="ps_st", bufs=4, space="PSUM") as ps_st,
            tc.tile_pool(name="ps_agg", bufs=1, space="PSUM") as ps_agg,
        ):
            ones = consts.tile([P, 1], F32)
            nc.vector.memset(ones, 1.0)

            wq_sb = consts.tile([P, ncc, E], PRJ_DT, tag="wq")
            wk_sb = consts.tile([P, ncc, E], PRJ_DT, tag="wk")
            wv_sb = consts.tile([P, ncc, E], PRJ_DT, tag="wv")
            nc.sync.dma_start(out=wq_sb, in_=wqT[:, :].rearrange("(c p) e -> p c e", p=P))
            nc.sync.dma_start(out=wk_sb, in_=wkT[:, :].rearrange("(c p) e -> p c e", p=P))
            nc.sync.dma_start(out=wv_sb, in_=wvT[:, :].rearrange("(c p) e -> p c e", p=P))
            bq_sb = consts.tile([E, 1], F32, tag="bq")
            bk_sb = consts.tile([E, 1], F32, tag="bk")
            nc.sync.dma_start(out=bq_sb, in_=bq[:, :])
            nc.sync.dma_start(out=bk_sb, in_=bk[:, :])
            bv_sb = consts.tile([P, E], F32, tag="bv")
            nc.sync.dma_start(out=bv_sb, in_=bv[:, :].to_broadcast((P, E)))

            QT_sb = persist.tile([E, N], QK_DT, tag="QT")  # [e, n] 2 heads x 64
            KT_sb = persist.tile([E, N], QK_DT, tag="KT")
            V_sb = persist.tile([P, NKT, E], AT_DT, tag="V")  # [k%128, kt, e]

            # ---- phase 0: projections --------------------------------------
            with tc.tile_pool(name="qkv", bufs=1) as qkv:
                qts = [qkv.tile([P, N], PRJ_DT, name=f"q{c}", tag=f"q{c}") for c in range(ncc)]
                kts = [qkv.tile([P, N], PRJ_DT, name=f"k{c}", tag=f"k{c}") for c in range(ncc)]
                vts = [qkv.tile([P, N], PRJ_DT, name=f"v{c}", tag=f"v{c}") for c in range(ncc)]
                for c in range(ncc):
                    sl = slice(c * P, (c + 1) * P)
                    nc.sync.dma_start(out=qts[c], in_=qT[sl, :])
                    nc.sync.dma_start(out=kts[c], in_=kT[sl, :])
                    nc.sync.dma_start(out=vts[c], in_=vT[sl, :])

                # Q^T and K^T: [e, n] = sum_c w[c, e] * xT[c, n]
                for dst, w_sb, b_sb, srcs in (
                    (QT_sb, wq_sb, bq_sb, qts),
                    (KT_sb, wk_sb, bk_sb, kts),
                ):
                    for nch in range(N // HF):
                        cols = slice(nch * HF, (nch + 1) * HF)
                        ps = ps_st.tile([P, HF], F32, tag="st", name=f"psp{nch}")
                        for c in range(ncc):
                            nc.tensor.matmul(
                                ps,
                                w_sb[:, c, :],
                                srcs[c][:, cols],
                                start=(c == 0),
                                stop=(c == ncc - 1),
                            )
                        nc.scalar.activation(
                            dst[:, cols],
                            ps,
                            mybir.ActivationFunctionType.Identity,
                            bias=b_sb,
                        )

                # V natural: [n, e] = sum_c vT[c, n] * w[c, e]
                for t in range(NKT):
                    ps = ps_st.tile([P, HF], F32, tag="st", name=f"psv{t}")
                    for c in range(ncc):
                        nc.tensor.matmul(
                            ps[:, :E],
                            vts[c][:, t * P : (t + 1) * P],
                            wv_sb[:, c, :],
                            start=(c == 0),
                            stop=(c == ncc - 1),
                        )
                    nc.vector.tensor_add(V_sb[:, t, :], ps[:, :E], bv_sb)

            # ---- phase 1: attention (software-pipelined emission) ---------
            with (
                tc.tile_pool(name="et", bufs=2) as etp,
                tc.tile_pool(name="at", bufs=2) as atp,
                tc.tile_pool(name="eacc", bufs=2) as eaccp,
                tc.tile_pool(name="mask", bufs=4) as maskp,
                tc.tile_pool(name="small", bufs=2) as smallp,
                tc.tile_pool(name="outp", bufs=2) as outp,
            ):

                def emit_st(qc, kt, eaccs):
                    """S^T + exp + mask-mult + E-sum accumulate for one k tile."""
                    mt = maskp.tile(
                        [P, HPC, QC], F32, tag="mt", name=f"mt_{qc}_{kt}"
                    )
                    mask_ap = bass.AP(
                        tensor=maskT,
                        offset=kt * P * N + qc * QC,
                        ap=[[N, P], [N * N, HPC], [1, QC]],
                    )
                    nc.sync.dma_start(out=mt, in_=mask_ap)
                    tiles = []
                    kcols = slice(kt * P, (kt + 1) * P)
                    for half in range(NHF):
                        for lh in range(HPC):
                            hsl = slice(lh * HD, (lh + 1) * HD)
                            rcols = slice(
                                qc * QC + half * HF, qc * QC + (half + 1) * HF
                            )
                            st = ps_st.tile(
                                [P, HF], F32, tag="st", name=f"st_{qc}_{kt}_{lh}_{half}"
                            )
                            nc.tensor.matmul(
                                st,
                                KT_sb[hsl, kcols],
                                QT_sb[hsl, rcols],
                                start=True,
                                stop=True,
                                tile_position=(lh * HD, 0),
                            )
                            et = etp.tile(
                                [P, HF], F32, tag=f"et{lh}{half}",
                                name=f"et_{qc}_{kt}_{lh}_{half}",
                            )
                            nc.scalar.activation(
                                et, st, mybir.ActivationFunctionType.Exp, scale=SCALING
                            )
                            ea = eaccs[lh * NHF + half]
                            if kt == 0:
                                nc.vector.tensor_copy(ea, et)
                            else:
                                nc.vector.tensor_add(ea, ea, et)
                            at = atp.tile(
                                [P, HF], AT_DT, tag=f"at{lh}{half}",
                                name=f"at_{qc}_{kt}_{lh}_{half}",
                            )
                            nc.vector.tensor_mul(
                                at, et, mt[:, lh, half * HF : (half + 1) * HF]
                            )
                            tiles.append((lh, half, at))
                    return tiles

                def emit_av(kt, tiles, aggs):
                    first, last = kt == 0, kt == NKT - 1
                    for lh, half, at in tiles:
                        esl = slice(lh * HD, (lh + 1) * HD)
                        hcols = slice(half * HF, (half + 1) * HF)
                        nc.tensor.matmul(
                            aggs[lh][:, hcols],
                            V_sb[:, kt, esl],
                            at,
                            start=first,
                            stop=last,
                            skip_group_check=True,
                        )

                for qc in range(NQC):
                    qcols = slice(qc * QC, (qc + 1) * QC)
                    aggs = [
                        ps_agg.tile([HD, QC], F32, tag=f"agg{lh}", name=f"agg_{qc}_{lh}")
                        for lh in range(HPC)
                    ]
                    eaccs = [
                        eaccp.tile(
                            [P, HF], F32, tag=f"ea{i}", name=f"ea_{qc}_{i}"
                        )
                        for i in range(HPC * NHF)
                    ]
                    tiles = emit_st(qc, 0, eaccs)
                    for kt in range(1, NKT):
                        nxt = emit_st(qc, kt, eaccs)
                        emit_av(kt - 1, tiles, aggs)
                        tiles = nxt
                    emit_av(NKT - 1, tiles, aggs)

                    # Z = ones^T @ E_acc (partition-direction sum), one matmul
                    # per (head, half); head lh's row lands at partition 32*lh.
                    zsb = smallp.tile([33, QC], F32, tag="zsb", name=f"zsb{qc}")
                    for lh in range(HPC):
                        zrow = lh * 32
                        for half in range(NHF):
                            zp = ps_st.tile(
                                [33, HF], F32, tag="st", name=f"zp_{qc}_{lh}_{half}"
                            )
                            nc.tensor.matmul(
                                zp[zrow : zrow + 1, :],
                                ones,
                                eaccs[lh * NHF + half],
                                start=True,
                                stop=True,
                                tile_position=(0, zrow),
                            )
                            nc.vector.tensor_copy(
                                zsb[zrow : zrow + 1, half * HF : (half + 1) * HF],
                                zp[zrow : zrow + 1, :],
                            )
                        nc.sync.dma_start(
                            out=zout[lh, qcols], in_=zsb[zrow : zrow + 1, :]
                        )
                    for lh in range(HPC):
                        osb = outp.tile(
                            [HD, QC], F32, tag=f"osb{lh}", name=f"osb_{qc}_{lh}"
                        )
                        nc.vector.tensor_copy(osb, aggs[lh])
                        nc.sync.dma_start(
                            out=outT[lh * HD : (lh + 1) * HD, qcols], in_=osb
                        )

    nc.compile()
    return nc


# ---------------------------------------------------------------------------
# Host side
# ---------------------------------------------------------------------------
def _prep_in_maps(q, k, v, mask_head, pearson_matrix, Wq, bq, Wk, bk, Wv, bv):
    f = np.float32
    q = np.asarray(q, f)
    k = np.asarray(k, f)
    v = np.asarray(v, f)
    mask_head = np.asarray(mask_head, f)
    Wq = np.asarray(Wq, f)
    Wk = np.asarray(Wk, f)
    Wv = np.asarray(Wv, f)
    bq = np.asarray(bq, f)
    bk = np.asarray(bk, f)
    bv = np.asarray(bv, f)

    # Only the diagonal of pearson is used by the computation.
    pm = np.asarray(pearson_matrix)
    diag = np.ascontiguousarray(np.diagonal(pm, axis1=-2, axis2=-1)).astype(f)

    qT = [_tcopy(q[b]) for b in range(B)]
    kTt = [_tcopy(k[b]) for b in range(B)]
    vTt = [_tcopy(v[b]) for b in range(B)]

    # maskT_all[b, h, k, q] = mask[b, h, q, k] * diag[b, h, k]; per-core masks
    # are contiguous zero-copy views maskT_all[b, h0:h0+HPC].
    maskT_all = _alloc((B, H, N, N), f)
    for b in range(B):
        for h in range(H):
            np.multiply(mask_head[b, h].T, diag[b, h][:, None], out=maskT_all[b, h])

    in_maps = []
    for c in range(NCORES):
        b = c // (NCORES // B)
        h0 = HPC * (c % (NCORES // B))
        esl = slice(h0 * HD, (h0 + HPC) * HD)
        in_maps.append(
            {
                "qT": qT[b],
                "kT": kTt[b],
                "vT": vTt[b],
                "wqT": _tcopy(Wq[esl, :]),
                "wkT": _tcopy(Wk[esl, :]),
                "wvT": _tcopy(Wv[esl, :]),
                "bq": np.ascontiguousarray(bq[esl]).reshape(E, 1),
                "bk": np.ascontiguousarray(bk[esl]).reshape(E, 1),
                "bv": np.ascontiguousarray(bv[esl]).reshape(1, E),
                "maskT": maskT_all[b, h0 : h0 + HPC],
            }
        )
    return in_maps


_NC_CACHE = None
LAST_RESULT = None  # BassKernelResults of the most recent run (for profiling)


def kernel(**inputs) -> np.ndarray:
    global _NC_CACHE, LAST_RESULT
    _install_shims()
    from concourse.bass_utils import run_bass_kernel_spmd

    if _NC_CACHE is None:
        _NC_CACHE = build_nc()
    nc = _NC_CACHE

    in_maps = _prep_in_maps(**inputs)

    trace = bool(int(os.environ.get("KERNEL_TRACE", "0")))
    kwargs = {}
    if trace:
        kwargs["trace"] = True
        tmpdir = os.environ.get("KERNEL_TRACE_DIR")
        if tmpdir:
            kwargs["tmpdir"] = tmpdir
    res = run_bass_kernel_spmd(nc, in_maps, list(range(NCORES)), **kwargs)
    LAST_RESULT = res

    out = _alloc((B, N, D), np.float32)
    for c in range(NCORES):
        b = c // (NCORES // B)
        h0 = HPC * (c % (NCORES // B))
        aggT = res.results[c]["outT"]  # (E, N) unnormalized
        z = res.results[c]["zout"]  # (HPC, N)
        out[b, :, h0 * HD : (h0 + HPC) * HD] = (
            aggT / np.repeat(z, HD, axis=0)
        ).T
    return out



# revision 3
# speedup vs baseline: 1.1492x; 1.1492x over previous
"""MultiHeadCrossAttention kernel for 8 Trainium2 NeuronCores.

Reference computation (b=2, nq=nk=2048, d_model=512, h=8, hd=64):
    Q = split_heads(q @ Wq.T + bq); K, V likewise
    S = Q K^T * hd^-0.5 ; A = softmax(S, -1) * mask_head * diag(pearson)
    out = merge_heads(A @ V)

Sharding: 16 (batch, head) pairs -> 2 heads of one batch per core.

Only the *diagonal* of pearson_matrix is used, so it is extracted on the
host and folded into the mask.  The mask is transposed, diag-folded,
tiled to the exact per-iteration consumption order and cast to fp16 on
the host, so every device-side mask DMA is one contiguous 256 KiB read.

All matmul operands are fp16 (PE runs 16-bit at 4x the fp32 rate, and
fp16's 10 mantissa bits keep the error ~1e-3, far under the 2e-2 gate).
PSUM accumulation stays fp32.

Device layout per core ("k on partitions, q on free axis"):
    S^T[k,q]   = sum_d K^T[d,k] Q^T[d,q]       (TensorE, d=64, 2 heads
                                                row-packed in the array)
    E^T        = exp(SCALING * S^T)            (ScalarE, PSUM->SBUF fp16)
    Z[q]      += ones^T @ E^T                  (TensorE, PSUM-accumulated
                                                across k tiles)
    A^T        = E^T * maskT_folded            (VectorE, fp16 2x mode)
    agg^T[e,q]+= sum_k V[k,e] A^T[k,q]         (TensorE, PSUM-accumulated)

The device returns unnormalized agg^T (128 rows = 2 heads x 64 dims) and
Z; the host divides, transposes and concatenates the 8 per-core slices.
"""

import ctypes
import os
import sys
import types

import numpy as np

import concourse.bacc as bacc
import concourse.bass as bass
import concourse.tile as tile
from concourse import mybir
from concourse.vector_clock import ScopedClock

F32 = mybir.dt.float32
F16 = mybir.dt.float16

B = 2
H = 8
N = 2048  # nq == nk
D = 512
HD = 64
HPC = 2  # heads per core
E = HPC * HD  # 128 output dims per core
SCALING = HD ** (-0.5)
NCORES = 8
P = 128
QC = 1024  # q super-chunk (2 per core)
NQC = N // QC
NKT = N // P  # 16 k tiles
HF = 512  # matmul free-dim chunk (one PSUM bank)
NCC = D // P  # 4 contraction chunks for the projections


# ---------------------------------------------------------------------------
# Page faults are extremely slow in this sandbox (~ms each); MAP_POPULATE
# prefaults an allocation in one syscall, ~100x faster for big arrays.
# ---------------------------------------------------------------------------
_libc = ctypes.CDLL(None, use_errno=True)
_libc.mmap.restype = ctypes.c_void_p
_libc.mmap.argtypes = [
    ctypes.c_void_p,
    ctypes.c_size_t,
    ctypes.c_int,
    ctypes.c_int,
    ctypes.c_int,
    ctypes.c_long,
]


def _alloc(shape, dtype=np.float32):
    nbytes = int(np.prod(shape)) * np.dtype(dtype).itemsize
    nbytes = (nbytes + 4095) & ~4095
    p = _libc.mmap(None, nbytes, 0x3, 0x02 | 0x20 | 0x8000, -1, 0)  # RW, PRIV|ANON|POPULATE
    if p in (None, ctypes.c_void_p(-1).value):
        return np.empty(shape, dtype)
    buf = (ctypes.c_byte * nbytes).from_address(p)
    return np.frombuffer(buf, dtype=dtype, count=int(np.prod(shape))).reshape(shape)


def _tcopy16(src):
    """Contiguous fp16 transposed copy of a 2-D array into prefaulted memory."""
    dst = _alloc((src.shape[1], src.shape[0]), np.float16)
    np.copyto(dst, src.T)
    return dst


# ---------------------------------------------------------------------------
# Environment shim: walrus in this container rejects >1 sync wait on
# CTRL-class instructions (NoOp/Drain), but TileContext's kernel-tail drain
# carries one wait per live semaphore.  Re-emit them as individual wait_ge
# instructions (one wait each) before a bare drain.
# ---------------------------------------------------------------------------
def _drain_and_barrier(self, tick_clock, wait_clock):
    probe = mybir.InstNoOp(
        name="wait_probe", ins=[], outs=[], engine=mybir.EngineType.SP
    )
    wait_clock.add_sem_waits(probe, ScopedClock({None: tick_clock.global_clock}))
    waits = list(probe.sync_info.on_wait) if probe.sync_info else []
    allocated = self.sems.allocated()
    by_name = {}
    for k, h in allocated.items():
        by_name[getattr(h, "name", str(k))] = h
    for w in waits:
        h = by_name.get(w.ant_name)
        assert h is not None, (w.ant_name, sorted(by_name))
        self.nc.sync.wait_ge(h, w.wait_value)
    self.nc.sync.drain()
    self.nc.all_engine_barrier()
    popped = self.nc._tile_sem_poison_stack.pop()
    assert popped is self._sem_poison
    self.nc.clear_and_free_semaphores(list(allocated.values()))
    self.nc.all_engine_barrier()


def _install_shims():
    tile.TileContext._drain_and_barrier = _drain_and_barrier
    if "antenv.axon_hooks" not in sys.modules:
        try:
            from trn_agent_boot.trn_boot import _ntff_profile_via_ctypes

            mod = types.ModuleType("antenv.axon_hooks")
            hook = _ntff_profile_via_ctypes("/opt/axon/libaxon_pjrt.so")
            mod.get_axon_ntff_profile_hook = lambda: hook
            mod.set_axon_ntff_profile_hook = lambda h: None
            sys.modules["antenv.axon_hooks"] = mod
        except Exception:
            pass


# ---------------------------------------------------------------------------
# Device kernel (one Bass program, SPMD over 8 cores; shards via in_maps)
# ---------------------------------------------------------------------------
def build_nc() -> bass.Bass:
    nc = bacc.Bacc("TRN2", target_bir_lowering=False, debug=False)

    qT = nc.dram_tensor("qT", [D, N], F16, kind="ExternalInput")
    kT = nc.dram_tensor("kT", [D, N], F16, kind="ExternalInput")
    vT = nc.dram_tensor("vT", [D, N], F16, kind="ExternalInput")
    wqT = nc.dram_tensor("wqT", [D, E], F16, kind="ExternalInput")
    wkT = nc.dram_tensor("wkT", [D, E], F16, kind="ExternalInput")
    wvT = nc.dram_tensor("wvT", [D, E], F16, kind="ExternalInput")
    bq = nc.dram_tensor("bq", [E, 1], F32, kind="ExternalInput")
    bk = nc.dram_tensor("bk", [E, 1], F32, kind="ExternalInput")
    bv = nc.dram_tensor("bv", [1, E], F32, kind="ExternalInput")
    # maskt[qc, kt, k, lh, q] = mask[b, h0+lh, qc*QC+q, kt*P+k]
    #                          * diag(pearson)[b, h0+lh, kt*P+k]
    maskt = nc.dram_tensor("maskt", [NQC, NKT, P, HPC, QC], F16, kind="ExternalInput")
    outT = nc.dram_tensor("outT", [E, N], F32, kind="ExternalOutput")
    # softmax denominators, normalization happens on the host
    zout = nc.dram_tensor("zout", [HPC, N], F32, kind="ExternalOutput")

    with tile.TileContext(nc) as tc:
        with (
            tc.tile_pool(name="consts", bufs=1) as consts,
            tc.tile_pool(name="persist", bufs=1) as persist,
            tc.tile_pool(name="ps", bufs=1, space="PSUM") as ps,
        ):
            ones = consts.tile([P, 1], F16)
            nc.vector.memset(ones, 1.0)
            scratch = consts.tile([1, 8], F32, tag="scratch")
            nc.vector.memset(scratch, 0.0)

            wq_sb = consts.tile([P, NCC, E], F16, tag="wq")
            wk_sb = consts.tile([P, NCC, E], F16, tag="wk")
            wv_sb = consts.tile([P, NCC, E], F16, tag="wv")
            nc.sync.dma_start(out=wq_sb, in_=wqT[:, :].rearrange("(c p) e -> p c e", p=P))
            nc.sync.dma_start(out=wk_sb, in_=wkT[:, :].rearrange("(c p) e -> p c e", p=P))
            nc.sync.dma_start(out=wv_sb, in_=wvT[:, :].rearrange("(c p) e -> p c e", p=P))
            bq_sb = consts.tile([E, 1], F32, tag="bq")
            bk_sb = consts.tile([E, 1], F32, tag="bk")
            nc.sync.dma_start(out=bq_sb, in_=bq[:, :])
            nc.sync.dma_start(out=bk_sb, in_=bk[:, :])
            bv_sb = consts.tile([P, E], F32, tag="bv")
            nc.sync.dma_start(out=bv_sb, in_=bv[:, :].to_broadcast((P, E)))

            # Preload the exp spline table while the qkv DMAs are in flight.
            nc.scalar.activation(scratch, scratch, mybir.ActivationFunctionType.Exp)

            QT_sb = persist.tile([E, N], F16, tag="QT")  # [e, n] 2 heads x 64
            KT_sb = persist.tile([E, N], F16, tag="KT")
            V_sb = persist.tile([P, NKT, E], F16, tag="V")  # [k%128, kt, e]

            # PSUM layout: 16 KiB/partition exactly.
            #   s0, s1 : per-head S^T tiles  [128, 1024] f32 (2 banks each)
            #   agg    : A^T @ V accumulator [128, 1024] f32 (heads col-packed)
            #   z      : softmax denominators (head lh at partition 32*lh)
            def s_tile(lh, name):
                return ps.tile([P, QC], F32, tag=f"s{lh}", name=name)

            def agg_tile(name):
                return ps.tile([P, QC], F32, tag="agg", name=name)

            def z_tile(name):
                return ps.tile([33, QC], F32, tag="z", name=name)

            # ---- phase 0: projections --------------------------------------
            with tc.tile_pool(name="qkv", bufs=1) as qkv:
                kts = [qkv.tile([P, N], F16, name=f"k{c}", tag=f"k{c}") for c in range(NCC)]
                qts = [qkv.tile([P, N], F16, name=f"q{c}", tag=f"q{c}") for c in range(NCC)]
                vts = [qkv.tile([P, N], F16, name=f"v{c}", tag=f"v{c}") for c in range(NCC)]
                for c in range(NCC):
                    sl = slice(c * P, (c + 1) * P)
                    nc.sync.dma_start(out=kts[c], in_=kT[sl, :])
                for c in range(NCC):
                    sl = slice(c * P, (c + 1) * P)
                    nc.sync.dma_start(out=qts[c], in_=qT[sl, :])
                for c in range(NCC):
                    sl = slice(c * P, (c + 1) * P)
                    nc.sync.dma_start(out=vts[c], in_=vT[sl, :])

                # K^T then Q^T: [e, n] = sum_c w[c, e] * xT[c, n].  Bias is
                # added during PSUM->SBUF eviction on VectorE (per-partition
                # scalar operand) to keep ScalarE free for the exp stream.
                for dst, w_sb, b_sb, srcs, nm in (
                    (KT_sb, wk_sb, bk_sb, kts, "k"),
                    (QT_sb, wq_sb, bq_sb, qts, "q"),
                ):
                    for j in range(N // QC):
                        pst = s_tile(j % 2, f"psp_{nm}{j}")
                        for half in range(QC // HF):
                            cols = slice(j * QC + half * HF, j * QC + (half + 1) * HF)
                            for c in range(NCC):
                                nc.tensor.matmul(
                                    pst[:, half * HF : (half + 1) * HF],
                                    w_sb[:, c, :],
                                    srcs[c][:, cols],
                                    start=(c == 0),
                                    stop=(c == NCC - 1),
                                )
                        nc.vector.tensor_scalar_add(
                            dst[:, j * QC : (j + 1) * QC], pst, b_sb
                        )

                # V natural layout: [n, e] = sum_c vT[c, n] * w[c, e].
                # 8 k-tiles per PSUM round (2 rounds through the s tags).
                for rnd in range(2):
                    pst = s_tile(rnd, f"psv{rnd}")
                    for t8 in range(8):
                        t = rnd * 8 + t8
                        for c in range(NCC):
                            nc.tensor.matmul(
                                pst[:, t8 * E : (t8 + 1) * E],
                                vts[c][:, t * P : (t + 1) * P],
                                wv_sb[:, c, :],
                                start=(c == 0),
                                stop=(c == NCC - 1),
                            )
                    for t8 in range(8):
                        t = rnd * 8 + t8
                        nc.vector.tensor_add(
                            V_sb[:, t, :], pst[:, t8 * E : (t8 + 1) * E], bv_sb
                        )

            # ---- phase 1: attention (software-pipelined emission) ---------
            with (
                tc.tile_pool(name="et", bufs=4) as etp,
                tc.tile_pool(name="at", bufs=4) as atp,
                tc.tile_pool(name="mask", bufs=3) as maskp,
                tc.tile_pool(name="small", bufs=2) as smallp,
                tc.tile_pool(name="outp", bufs=2) as outp,
            ):

                def emit_s(qc, kt, lh, s_ps):
                    """S^T matmuls for one (q chunk, k tile, head)."""
                    kcols = slice(kt * P, (kt + 1) * P)
                    hsl = slice(lh * HD, (lh + 1) * HD)
                    for half in range(QC // HF):
                        rcols = slice(qc * QC + half * HF, qc * QC + (half + 1) * HF)
                        nc.tensor.matmul(
                            s_ps[:, half * HF : (half + 1) * HF],
                            KT_sb[hsl, kcols],
                            QT_sb[hsl, rcols],
                            start=True,
                            stop=True,
                            tile_position=(lh * HD, 0),
                        )

                for qc in range(NQC):
                    qcols = slice(qc * QC, (qc + 1) * QC)
                    agg = agg_tile(f"agg{qc}")
                    zps = z_tile(f"z{qc}")

                    # Software pipeline: emit S(kt=0) for both heads, then per
                    # k tile: exp/mask/Z/AV of kt alongside S of kt+1.
                    s_cur = [s_tile(lh, f"s_{qc}_0_{lh}") for lh in range(HPC)]
                    for lh in range(HPC):
                        emit_s(qc, 0, lh, s_cur[lh])

                    for kt in range(NKT):
                        first, last = kt == 0, kt == NKT - 1
                        mt = maskp.tile([P, HPC, QC], F16, tag="mt", name=f"mt_{qc}_{kt}")
                        nc.sync.dma_start(out=mt, in_=maskt[qc, kt])
                        et = etp.tile([P, HPC, QC], F16, tag="et", name=f"et_{qc}_{kt}")
                        at = atp.tile([P, HPC, QC], F16, tag="at", name=f"at_{qc}_{kt}")
                        s_nxt = (
                            [s_tile(lh, f"s_{qc}_{kt + 1}_{lh}") for lh in range(HPC)]
                            if not last
                            else None
                        )
                        for lh in range(HPC):
                            nc.scalar.activation(
                                et[:, lh, :],
                                s_cur[lh],
                                mybir.ActivationFunctionType.Exp,
                                scale=SCALING,
                            )
                            # S^T for the next k tile reuses this head's PSUM
                            # banks; emit right after the exp that frees them.
                            if not last:
                                emit_s(qc, kt + 1, lh, s_nxt[lh])
                            nc.vector.tensor_mul(at[:, lh, :], et[:, lh, :], mt[:, lh, :])
                            esl = slice(lh * HD, (lh + 1) * HD)
                            for half in range(QC // HF):
                                hcols = slice(half * HF, (half + 1) * HF)
                                # Z accumulation: ones^T @ E^T, head lh's row
                                # lands at partition 32*lh.
                                nc.tensor.matmul(
                                    zps[lh * 32 : lh * 32 + 1, hcols],
                                    ones,
                                    et[:, lh, hcols],
                                    start=first,
                                    stop=last,
                                    tile_position=(0, lh * 32),
                                    skip_group_check=True,
                                )
                                nc.tensor.matmul(
                                    agg[esl, hcols],
                                    V_sb[:, kt, esl],
                                    at[:, lh, hcols],
                                    start=first,
                                    stop=last,
                                    tile_position=(0, lh * HD),
                                    skip_group_check=True,
                                )
                        s_cur = s_nxt

                    zsb = smallp.tile([33, QC], F32, tag="zsb", name=f"zsb{qc}")
                    nc.vector.tensor_copy(zsb, zps)
                    for lh in range(HPC):
                        nc.sync.dma_start(
                            out=zout[lh, qcols], in_=zsb[lh * 32 : lh * 32 + 1, :]
                        )
                    osb = outp.tile([P, QC], F32, tag="osb", name=f"osb_{qc}")
                    nc.vector.tensor_copy(osb, agg)
                    nc.sync.dma_start(out=outT[:, qcols], in_=osb)

    nc.compile()
    return nc


# ---------------------------------------------------------------------------
# Host side
# ---------------------------------------------------------------------------
def _prep_in_maps(q, k, v, mask_head, pearson_matrix, Wq, bq, Wk, bk, Wv, bv):
    f = np.float32
    q = np.asarray(q, f)
    k = np.asarray(k, f)
    v = np.asarray(v, f)
    mask_head = np.asarray(mask_head, f)
    Wq = np.asarray(Wq, f)
    Wk = np.asarray(Wk, f)
    Wv = np.asarray(Wv, f)
    bq = np.asarray(bq, f).reshape(D)
    bk = np.asarray(bk, f).reshape(D)
    bv = np.asarray(bv, f).reshape(D)

    # Only the diagonal of pearson is used by the computation.
    pm = np.asarray(pearson_matrix)
    diag = np.ascontiguousarray(np.diagonal(pm, axis1=-2, axis2=-1)).astype(f)

    qT = [_tcopy16(q[b]) for b in range(B)]
    kTt = [_tcopy16(k[b]) for b in range(B)]
    vTt = [_tcopy16(v[b]) for b in range(B)]

    # Per-(b,h) mask, transposed to [k, q], diag-folded, tiled to the exact
    # per-iteration consumption order: [qc, kt, k, lh, q].
    maskt_all = _alloc((B, H // HPC, NQC, NKT, P, HPC, QC), np.float16)
    for b in range(B):
        for h in range(H):
            md = mask_head[b, h].T * diag[b, h][:, None]  # [k, q] f32
            tiled = md.reshape(NKT, P, NQC, QC).transpose(2, 0, 1, 3)
            maskt_all[b, h // HPC, :, :, :, h % HPC, :] = tiled

    in_maps = []
    for c in range(NCORES):
        b = c // (NCORES // B)
        h0 = HPC * (c % (NCORES // B))
        esl = slice(h0 * HD, (h0 + HPC) * HD)
        in_maps.append(
            {
                "qT": qT[b],
                "kT": kTt[b],
                "vT": vTt[b],
                "wqT": _tcopy16(Wq[esl, :]),
                "wkT": _tcopy16(Wk[esl, :]),
                "wvT": _tcopy16(Wv[esl, :]),
                "bq": np.ascontiguousarray(bq[esl]).reshape(E, 1),
                "bk": np.ascontiguousarray(bk[esl]).reshape(E, 1),
                "bv": np.ascontiguousarray(bv[esl]).reshape(1, E),
                "maskt": maskt_all[b, h0 // HPC],
            }
        )
    return in_maps


_NC_CACHE = None
LAST_RESULT = None  # BassKernelResults of the most recent run (for profiling)


def kernel(**inputs) -> np.ndarray:
    global _NC_CACHE, LAST_RESULT
    _install_shims()
    from concourse.bass_utils import run_bass_kernel_spmd

    if _NC_CACHE is None:
        _NC_CACHE = build_nc()
    nc = _NC_CACHE

    in_maps = _prep_in_maps(**inputs)

    trace = bool(int(os.environ.get("KERNEL_TRACE", "0")))
    kwargs = {}
    if trace:
        kwargs["trace"] = True
        tmpdir = os.environ.get("KERNEL_TRACE_DIR")
        if tmpdir:
            kwargs["tmpdir"] = tmpdir
    res = run_bass_kernel_spmd(nc, in_maps, list(range(NCORES)), **kwargs)
    LAST_RESULT = res

    out = _alloc((B, N, D), np.float32)
    for c in range(NCORES):
        b = c // (NCORES // B)
        h0 = HPC * (c % (NCORES // B))
        aggT = res.results[c]["outT"]  # (E, N) unnormalized
        z = res.results[c]["zout"]  # (HPC, N)
        out[b, :, h0 * HD : (h0 + HPC) * HD] = (
            aggT / np.repeat(z, HD, axis=0)
        ).T
    return out


# revision 4
# speedup vs baseline: 2.0972x; 1.8249x over previous
"""MultiHeadCrossAttention kernel for 8 Trainium2 NeuronCores.

Reference computation (b=2, nq=nk=2048, d_model=512, h=8, hd=64):
    Q = split_heads(q @ Wq.T + bq); K, V likewise
    S = Q K^T * hd^-0.5 ; A = softmax(S, -1) * mask_head * diag(pearson)
    out = merge_heads(A @ V)

Sharding: 16 (batch, head) pairs -> 2 heads of one batch per core.

Only the *diagonal* of pearson_matrix is used, so it is extracted on the
host and folded into the mask.  The mask is transposed, diag-folded,
tiled to the exact per-iteration consumption order and cast to fp16 on
the host, so every device-side mask DMA is one contiguous 512 KiB read.

All matmul operands are fp16 (PE runs 16-bit at 4x the fp32 rate, and
fp16's 10 mantissa bits keep the error ~1e-3, far under the 2e-2 gate).
PSUM accumulation stays fp32.

Device layout per core ("k on partitions, q on free axis"):
    S^T[k,q]   = sum_d K^T[d,k] Q^T[d,q]       (TensorE, d=64 contraction)
    E^T        = exp(SCALING * S^T)            (ScalarE, PSUM->SBUF fp16)
    Z[q]      += ones^T @ E^T                  (TensorE, PSUM-accumulated,
                                                heads packed in col groups
                                                0/32 -> run concurrently)
    A^T        = E^T * maskT_folded            (VectorE, fp16 2x mode)
    agg^T[e,q]+= sum_k V[k,e] A^T[k,q]         (TensorE, PSUM-accumulated,
                                                heads col-packed 0/64)

PSUM (16 KiB/partition, exactly 8 banks):
    s0, s1 : per-head S^T staging [128,1024] f32 (2 banks each)
    agg    : [128,1024] f32, heads col-packed   (2 banks)
    z      : [33,1024] f32, head lh at row 32lh (2 banks)

The per-head S slots double-buffer the exp stream: the S matmuls for
k-tile kt+1 of head h are emitted right after head h's exp of k-tile kt,
so ScalarE (the bottleneck engine at ~2.8us/k-tile) never waits on PE.
Z/AV matmuls for both heads are emitted as adjacent pairs targeting
disjoint PE column groups, so each pair streams concurrently through the
array.

The device returns unnormalized agg^T (128 rows = 2 heads x 64 dims) and
Z; the host divides, transposes and concatenates the 8 per-core slices.
"""

import ctypes
import os
import sys
import types

import numpy as np

import concourse.bacc as bacc
import concourse.bass as bass
import concourse.tile as tile
from concourse import mybir
from concourse.vector_clock import ScopedClock

F32 = mybir.dt.float32
F16 = mybir.dt.float16

B = 2
H = 8
N = 2048  # nq == nk
D = 512
HD = 64
HPC = 2  # heads per core
E = HPC * HD  # 128 output dims per core
SCALING = HD ** (-0.5)
NCORES = 8
P = 128
QC = 1024  # q super-chunk (2 per core)
NQC = N // QC
NKT = N // P  # 16 k tiles
HF = 512  # matmul free-dim chunk (one PSUM bank)
NCC = D // P  # 4 contraction chunks for the projections


# ---------------------------------------------------------------------------
# Page faults are extremely slow in this sandbox (~ms each); MAP_POPULATE
# prefaults an allocation in one syscall, ~100x faster for big arrays.
# ---------------------------------------------------------------------------
_libc = ctypes.CDLL(None, use_errno=True)
_libc.mmap.restype = ctypes.c_void_p
_libc.mmap.argtypes = [
    ctypes.c_void_p,
    ctypes.c_size_t,
    ctypes.c_int,
    ctypes.c_int,
    ctypes.c_int,
    ctypes.c_long,
]


def _alloc(shape, dtype=np.float32):
    nbytes = int(np.prod(shape)) * np.dtype(dtype).itemsize
    nbytes = (nbytes + 4095) & ~4095
    p = _libc.mmap(None, nbytes, 0x3, 0x02 | 0x20 | 0x8000, -1, 0)  # RW, PRIV|ANON|POPULATE
    if p in (None, ctypes.c_void_p(-1).value):
        return np.empty(shape, dtype)
    buf = (ctypes.c_byte * nbytes).from_address(p)
    return np.frombuffer(buf, dtype=dtype, count=int(np.prod(shape))).reshape(shape)


def _tcopy16(src):
    """Contiguous fp16 transposed copy of a 2-D array into prefaulted memory."""
    dst = _alloc((src.shape[1], src.shape[0]), np.float16)
    np.copyto(dst, src.T)
    return dst


# ---------------------------------------------------------------------------
# Environment shim: walrus in this container rejects >1 sync wait on
# CTRL-class instructions (NoOp/Drain), but TileContext's kernel-tail drain
# carries one wait per live semaphore.  Re-emit them as individual wait_ge
# instructions (one wait each) before a bare drain.
# ---------------------------------------------------------------------------
def _drain_and_barrier(self, tick_clock, wait_clock):
    probe = mybir.InstNoOp(
        name="wait_probe", ins=[], outs=[], engine=mybir.EngineType.SP
    )
    wait_clock.add_sem_waits(probe, ScopedClock({None: tick_clock.global_clock}))
    waits = list(probe.sync_info.on_wait) if probe.sync_info else []
    allocated = self.sems.allocated()
    by_name = {}
    for k, h in allocated.items():
        by_name[getattr(h, "name", str(k))] = h
    for w in waits:
        h = by_name.get(w.ant_name)
        assert h is not None, (w.ant_name, sorted(by_name))
        self.nc.sync.wait_ge(h, w.wait_value)
    self.nc.sync.drain()
    self.nc.all_engine_barrier()
    popped = self.nc._tile_sem_poison_stack.pop()
    assert popped is self._sem_poison
    self.nc.clear_and_free_semaphores(list(allocated.values()))
    self.nc.all_engine_barrier()


def _install_shims():
    tile.TileContext._drain_and_barrier = _drain_and_barrier
    if "antenv.axon_hooks" not in sys.modules:
        try:
            from trn_agent_boot.trn_boot import _ntff_profile_via_ctypes

            mod = types.ModuleType("antenv.axon_hooks")
            hook = _ntff_profile_via_ctypes("/opt/axon/libaxon_pjrt.so")
            mod.get_axon_ntff_profile_hook = lambda: hook
            mod.set_axon_ntff_profile_hook = lambda h: None
            sys.modules["antenv.axon_hooks"] = mod
        except Exception:
            pass


# ---------------------------------------------------------------------------
# Device kernel (one Bass program, SPMD over 8 cores; shards via in_maps)
# ---------------------------------------------------------------------------
def build_nc() -> bass.Bass:
    nc = bacc.Bacc("TRN2", target_bir_lowering=False, debug=False)

    qT = nc.dram_tensor("qT", [D, N], F16, kind="ExternalInput")
    kT = nc.dram_tensor("kT", [D, N], F16, kind="ExternalInput")
    vT = nc.dram_tensor("vT", [D, N], F16, kind="ExternalInput")
    wqT = nc.dram_tensor("wqT", [D, E], F16, kind="ExternalInput")
    wkT = nc.dram_tensor("wkT", [D, E], F16, kind="ExternalInput")
    wvT = nc.dram_tensor("wvT", [D, E], F16, kind="ExternalInput")
    bq = nc.dram_tensor("bq", [E, 1], F32, kind="ExternalInput")
    bk = nc.dram_tensor("bk", [E, 1], F32, kind="ExternalInput")
    bv = nc.dram_tensor("bv", [1, E], F32, kind="ExternalInput")
    # maskt[qc, kt, k, lh, q] = mask[b, h0+lh, qc*QC+q, kt*P+k]
    #                          * diag(pearson)[b, h0+lh, kt*P+k]
    maskt = nc.dram_tensor("maskt", [NQC, NKT, P, HPC, QC], F16, kind="ExternalInput")
    outT = nc.dram_tensor("outT", [E, N], F32, kind="ExternalOutput")
    # softmax denominators, normalization happens on the host
    zout = nc.dram_tensor("zout", [HPC, N], F32, kind="ExternalOutput")

    with tile.TileContext(nc) as tc:
        with (
            tc.tile_pool(name="consts", bufs=1) as consts,
            tc.tile_pool(name="persist", bufs=1) as persist,
            tc.tile_pool(name="ps", bufs=1, space="PSUM") as ps,
        ):
            ones = consts.tile([P, 1], F16)
            nc.vector.memset(ones, 1.0)
            scratch = consts.tile([1, 8], F32, tag="scratch")
            nc.vector.memset(scratch, 0.0)

            wq_sb = consts.tile([P, NCC, E], F16, tag="wq")
            wk_sb = consts.tile([P, NCC, E], F16, tag="wk")
            wv_sb = consts.tile([P, NCC, E], F16, tag="wv")
            nc.sync.dma_start(out=wq_sb, in_=wqT[:, :].rearrange("(c p) e -> p c e", p=P))
            nc.sync.dma_start(out=wk_sb, in_=wkT[:, :].rearrange("(c p) e -> p c e", p=P))
            nc.sync.dma_start(out=wv_sb, in_=wvT[:, :].rearrange("(c p) e -> p c e", p=P))
            bq_sb = consts.tile([E, 1], F32, tag="bq")
            bk_sb = consts.tile([E, 1], F32, tag="bk")
            nc.sync.dma_start(out=bq_sb, in_=bq[:, :])
            nc.sync.dma_start(out=bk_sb, in_=bk[:, :])
            bv_sb = consts.tile([P, E], F32, tag="bv")
            nc.sync.dma_start(out=bv_sb, in_=bv[:, :].to_broadcast((P, E)))

            # Preload the exp spline table while the qkv DMAs are in flight.
            nc.scalar.activation(scratch, scratch, mybir.ActivationFunctionType.Exp)

            QT_sb = persist.tile([E, N], F16, tag="QT")  # [e, n] 2 heads x 64
            KT_sb = persist.tile([E, N], F16, tag="KT")
            V_sb = persist.tile([P, NKT, E], F16, tag="V")  # [k%128, kt, e]

            def s_tile(lh, name):
                return ps.tile([P, QC], F32, tag=f"s{lh}", name=name)

            def agg_tile(name):
                return ps.tile([P, QC], F32, tag="agg", name=name)

            def z_tile(name):
                return ps.tile([33, QC], F32, tag="z", name=name)

            # ---- phase 0: projections --------------------------------------
            with tc.tile_pool(name="qkv", bufs=1) as qkv:
                kts = [qkv.tile([P, N], F16, name=f"k{c}", tag=f"k{c}") for c in range(NCC)]
                qts = [qkv.tile([P, N], F16, name=f"q{c}", tag=f"q{c}") for c in range(NCC)]
                vts = [qkv.tile([P, N], F16, name=f"v{c}", tag=f"v{c}") for c in range(NCC)]
                for c in range(NCC):
                    sl = slice(c * P, (c + 1) * P)
                    nc.sync.dma_start(out=kts[c], in_=kT[sl, :])
                for c in range(NCC):
                    sl = slice(c * P, (c + 1) * P)
                    nc.sync.dma_start(out=qts[c], in_=qT[sl, :])
                for c in range(NCC):
                    sl = slice(c * P, (c + 1) * P)
                    nc.sync.dma_start(out=vts[c], in_=vT[sl, :])

                # K^T then Q^T: [e, n] = sum_c w[c, e] * xT[c, n].  Bias is
                # added during PSUM->SBUF eviction on VectorE (per-partition
                # scalar operand) to keep ScalarE free for the exp stream.
                for dst, w_sb, b_sb, srcs, nm in (
                    (KT_sb, wk_sb, bk_sb, kts, "k"),
                    (QT_sb, wq_sb, bq_sb, qts, "q"),
                ):
                    for j in range(N // QC):
                        pst = s_tile(j % 2, f"psp_{nm}{j}")
                        for half in range(QC // HF):
                            cols = slice(j * QC + half * HF, j * QC + (half + 1) * HF)
                            for c in range(NCC):
                                nc.tensor.matmul(
                                    pst[:, half * HF : (half + 1) * HF],
                                    w_sb[:, c, :],
                                    srcs[c][:, cols],
                                    start=(c == 0),
                                    stop=(c == NCC - 1),
                                )
                        nc.vector.tensor_scalar_add(
                            dst[:, j * QC : (j + 1) * QC], pst, b_sb
                        )

                # V natural layout: [n, e] = sum_c vT[c, n] * w[c, e].
                # 8 k-tiles per PSUM round (2 rounds through the s tags).
                for rnd in range(2):
                    pst = s_tile(rnd, f"psv{rnd}")
                    for t8 in range(8):
                        t = rnd * 8 + t8
                        for c in range(NCC):
                            nc.tensor.matmul(
                                pst[:, t8 * E : (t8 + 1) * E],
                                vts[c][:, t * P : (t + 1) * P],
                                wv_sb[:, c, :],
                                start=(c == 0),
                                stop=(c == NCC - 1),
                            )
                    for t8 in range(8):
                        t = rnd * 8 + t8
                        nc.vector.tensor_add(
                            V_sb[:, t, :], pst[:, t8 * E : (t8 + 1) * E], bv_sb
                        )

            # ---- phase 1: attention (software-pipelined emission) ---------
            with (
                tc.tile_pool(name="et", bufs=4) as etp,
                tc.tile_pool(name="at", bufs=4) as atp,
                tc.tile_pool(name="mask", bufs=3) as maskp,
                tc.tile_pool(name="small", bufs=2) as smallp,
                tc.tile_pool(name="outp", bufs=2) as outp,
            ):

                def emit_s(qc, kt, lh, s_ps):
                    """S^T matmuls for one (q chunk, k tile, head)."""
                    kcols = slice(kt * P, (kt + 1) * P)
                    hsl = slice(lh * HD, (lh + 1) * HD)
                    for half in range(QC // HF):
                        rcols = slice(qc * QC + half * HF, qc * QC + (half + 1) * HF)
                        nc.tensor.matmul(
                            s_ps[:, half * HF : (half + 1) * HF],
                            KT_sb[hsl, kcols],
                            QT_sb[hsl, rcols],
                            start=True,
                            stop=True,
                            tile_position=(lh * HD, 0),
                        )

                for qc in range(NQC):
                    qcols = slice(qc * QC, (qc + 1) * QC)
                    agg = agg_tile(f"agg{qc}")
                    zps = z_tile(f"z{qc}")

                    s_cur = [s_tile(lh, f"s_{qc}_0_{lh}") for lh in range(HPC)]
                    for lh in range(HPC):
                        emit_s(qc, 0, lh, s_cur[lh])

                    for kt in range(NKT):
                        first, last = kt == 0, kt == NKT - 1
                        mt = maskp.tile([P, HPC, QC], F16, tag="mt", name=f"mt_{qc}_{kt}")
                        nc.sync.dma_start(out=mt, in_=maskt[qc, kt])
                        et = etp.tile([P, HPC, QC], F16, tag="et", name=f"et_{qc}_{kt}")
                        at = atp.tile([P, HPC, QC], F16, tag="at", name=f"at_{qc}_{kt}")
                        s_nxt = (
                            [s_tile(lh, f"s_{qc}_{kt + 1}_{lh}") for lh in range(HPC)]
                            if not last
                            else None
                        )
                        for lh in range(HPC):
                            nc.scalar.activation(
                                et[:, lh, :],
                                s_cur[lh],
                                mybir.ActivationFunctionType.Exp,
                                scale=SCALING,
                            )
                            nc.vector.tensor_mul(at[:, lh, :], et[:, lh, :], mt[:, lh, :])
                            # S^T for the next k tile reuses this head's PSUM
                            # banks; emit right after the exp that frees them.
                            if not last:
                                emit_s(qc, kt + 1, lh, s_nxt[lh])

                        # Z and AV packs: both heads' matmuls emitted adjacent
                        # with disjoint PE column groups -> they stream through
                        # the array concurrently.
                        for half in range(QC // HF):
                            hcols = slice(half * HF, (half + 1) * HF)
                            for lh in range(HPC):
                                nc.tensor.matmul(
                                    zps[lh * 32 : lh * 32 + 1, hcols],
                                    ones,
                                    et[:, lh, hcols],
                                    start=first,
                                    stop=last,
                                    tile_position=(0, lh * 32),
                                    skip_group_check=True,
                                )
                        for half in range(QC // HF):
                            hcols = slice(half * HF, (half + 1) * HF)
                            for lh in range(HPC):
                                esl = slice(lh * HD, (lh + 1) * HD)
                                nc.tensor.matmul(
                                    agg[esl, hcols],
                                    V_sb[:, kt, esl],
                                    at[:, lh, hcols],
                                    start=first,
                                    stop=last,
                                    tile_position=(0, lh * HD),
                                    skip_group_check=True,
                                )
                        s_cur = s_nxt

                    zsb = smallp.tile([33, QC], F32, tag="zsb", name=f"zsb{qc}")
                    nc.vector.tensor_copy(zsb, zps)
                    for lh in range(HPC):
                        nc.sync.dma_start(
                            out=zout[lh, qcols], in_=zsb[lh * 32 : lh * 32 + 1, :]
                        )
                    osb = outp.tile([P, QC], F32, tag="osb", name=f"osb_{qc}")
                    nc.vector.tensor_copy(osb, agg)
                    nc.sync.dma_start(out=outT[:, qcols], in_=osb)

    nc.compile()
    return nc


# ---------------------------------------------------------------------------
# Host side
# ---------------------------------------------------------------------------
def _prep_in_maps(q, k, v, mask_head, pearson_matrix, Wq, bq, Wk, bk, Wv, bv):
    f = np.float32
    q = np.asarray(q, f)
    k = np.asarray(k, f)
    v = np.asarray(v, f)
    mask_head = np.asarray(mask_head, f)
    Wq = np.asarray(Wq, f)
    Wk = np.asarray(Wk, f)
    Wv = np.asarray(Wv, f)
    bq = np.asarray(bq, f).reshape(D)
    bk = np.asarray(bk, f).reshape(D)
    bv = np.asarray(bv, f).reshape(D)

    # Only the diagonal of pearson is used by the computation.
    pm = np.asarray(pearson_matrix)
    diag = np.ascontiguousarray(np.diagonal(pm, axis1=-2, axis2=-1)).astype(f)

    qT = [_tcopy16(q[b]) for b in range(B)]
    kTt = [_tcopy16(k[b]) for b in range(B)]
    vTt = [_tcopy16(v[b]) for b in range(B)]

    # Per-(b,h) mask, transposed to [k, q], diag-folded, tiled to the exact
    # per-iteration consumption order: [qc, kt, k, lh, q].
    maskt_all = _alloc((B, H // HPC, NQC, NKT, P, HPC, QC), np.float16)
    for b in range(B):
        for h in range(H):
            md = mask_head[b, h].T * diag[b, h][:, None]  # [k, q] f32
            tiled = md.reshape(NKT, P, NQC, QC).transpose(2, 0, 1, 3)
            maskt_all[b, h // HPC, :, :, :, h % HPC, :] = tiled

    in_maps = []
    for c in range(NCORES):
        b = c // (NCORES // B)
        h0 = HPC * (c % (NCORES // B))
        esl = slice(h0 * HD, (h0 + HPC) * HD)
        in_maps.append(
            {
                "qT": qT[b],
                "kT": kTt[b],
                "vT": vTt[b],
                "wqT": _tcopy16(Wq[esl, :]),
                "wkT": _tcopy16(Wk[esl, :]),
                "wvT": _tcopy16(Wv[esl, :]),
                "bq": np.ascontiguousarray(bq[esl]).reshape(E, 1),
                "bk": np.ascontiguousarray(bk[esl]).reshape(E, 1),
                "bv": np.ascontiguousarray(bv[esl]).reshape(1, E),
                "maskt": maskt_all[b, h0 // HPC],
            }
        )
    return in_maps


_NC_CACHE = None
LAST_RESULT = None  # BassKernelResults of the most recent run (for profiling)


def kernel(**inputs) -> np.ndarray:
    global _NC_CACHE, LAST_RESULT
    _install_shims()
    from concourse.bass_utils import run_bass_kernel_spmd

    if _NC_CACHE is None:
        _NC_CACHE = build_nc()
    nc = _NC_CACHE

    in_maps = _prep_in_maps(**inputs)

    trace = bool(int(os.environ.get("KERNEL_TRACE", "0")))
    kwargs = {}
    if trace:
        kwargs["trace"] = True
        tmpdir = os.environ.get("KERNEL_TRACE_DIR")
        if tmpdir:
            kwargs["tmpdir"] = tmpdir
    res = run_bass_kernel_spmd(nc, in_maps, list(range(NCORES)), **kwargs)
    LAST_RESULT = res

    out = _alloc((B, N, D), np.float32)
    for c in range(NCORES):
        b = c // (NCORES // B)
        h0 = HPC * (c % (NCORES // B))
        aggT = res.results[c]["outT"]  # (E, N) unnormalized
        z = res.results[c]["zout"]  # (HPC, N)
        out[b, :, h0 * HD : (h0 + HPC) * HD] = (
            aggT / np.repeat(z, HD, axis=0)
        ).T
    return out
